# revision 2
# baseline (speedup 1.0000x reference)
"""Cosine-similarity kernel (x[16384,512] vs weights[4096,512] -> [16384,4096])
on 8 Trainium2 NeuronCores, data-parallel over the x batch dim.

Per core: x shard [2048,512], full weights [4096,512], staged as fp16.
  out = normalize(x) @ normalize(w).T
Row norms are computed on ACT/DVE from the row-major tiles; normalization is
folded into the PE transposes by multiplying against diag(1/||row||) instead
of the identity, so both GEMM operands enter the matmul pre-normalized and
PSUM eviction is a pure copy. GEMM is m-outer over two column halves so the
output leaves in [128, 2048] row-chunk DMAs.
"""
import numpy as np

B, D, N = 16384, 512, 4096
NCORES = 8
BS = B // NCORES          # 2048 rows per core
MT = BS // 128            # 16 x row-tiles
NT = N // 128             # 32 w row-tiles
KC = D // 128             # 4 k-chunks
HALF = N // 2             # output column half

_cached = {}


def _build():
    import concourse.mybir as mybir
    import concourse.tile as tile
    from concourse import bacc
    from concourse.masks import make_identity

    F32, F16 = mybir.dt.float32, mybir.dt.float16
    AOP = mybir.AluOpType

    nc = bacc.Bacc(None, target_bir_lowering=False)
    x = nc.dram_tensor("x", [BS, D], F16, kind="ExternalInput")
    w = nc.dram_tensor("weights", [N, D], F16, kind="ExternalInput")
    o = nc.dram_tensor("out", [BS, N], F32, kind="ExternalOutput")

    with tile.TileContext(nc) as tc:
        with (
            tc.tile_pool(name="const", bufs=1) as const,
            tc.tile_pool(name="big", bufs=1) as big,
            tc.tile_pool(name="stage", bufs=3) as stage,
            tc.tile_pool(name="norm", bufs=6) as norm,
            tc.tile_pool(name="rowb", bufs=3) as rowb,
            tc.tile_pool(name="mmps", bufs=6, space="PSUM") as mmps,
            tc.tile_pool(name="trps", bufs=2, space="PSUM") as trps,
        ):
            ident = const.tile([128, 128], F16, name="ident")
            make_identity(nc, ident[:])

            # k-major transposed operands: [128(d%128), k, cols]
            wT = const.tile([128, KC * N], F16, name="wT")
            xT = const.tile([128, KC * BS], F16, name="xT")
            wT3 = wT[:].rearrange("p (k n) -> p k n", k=KC)
            xT3 = xT[:].rearrange("p (k n) -> p k n", k=KC)

            rr = [0]  # ACT/DVE round-robin for copies

            def copy_rr(dst, src):
                if rr[0] % 2 == 0:
                    nc.scalar.copy(dst, src)
                else:
                    nc.vector.tensor_copy(dst, src)
                rr[0] += 1

            def load4(src, row0):
                """One DMA pulling 4 consecutive 128-row tiles."""
                t4 = stage.tile([128, 4 * D], F16, name="t4", tag="ld")
                dst = t4.rearrange("p (g d) -> p g d", g=4)
                srcap = src[row0 : row0 + 4 * 128, :].rearrange(
                    "(g p) d -> p g d", p=128
                )
                nc.sync.dma_start(dst, srcap)
                return t4

            def prep(t4, g, dstT3, col0):
                """Normalize+transpose one 128-row tile into dstT3[:, :, col0:]."""
                t = t4[:, g * D : (g + 1) * D]
                sq = stage.tile([128, D], F16, name="sq", tag="sq")
                ss = norm.tile([128, 1], F32, name="ss", tag="ss")
                nc.vector.tensor_tensor_reduce(
                    out=sq[:], in0=t, in1=t, scale=1.0, scalar=0.0,
                    op0=AOP.mult, op1=AOP.add, accum_out=ss[:])
                inv = norm.tile([128, 1], F32, name="inv", tag="inv")
                nc.vector.reciprocal(inv[:], ss[:])
                rn = norm.tile([128, 1], F32, name="rn", tag="rn")
                nc.scalar.sqrt(rn[:], inv[:])
                dg = norm.tile([128, 128], F16, name="dg", tag="dg")
                nc.vector.tensor_scalar_mul(dg[:], ident[:], rn[:])
                pt = trps.tile([128, KC * 128], F32, name="pt", tag="pt")
                for k in range(KC):
                    # pt[:, k] = (t chunk k).T @ diag(rn): transposed+normalized
                    nc.tensor.matmul(
                        pt[:, k * 128 : (k + 1) * 128],
                        t[:, k * 128 : (k + 1) * 128], dg[:],
                        start=True, stop=True)
                copy_rr(
                    dstT3[:, :, col0 : col0 + 128],
                    pt[:].rearrange("p (k c) -> p k c", k=KC))

            def prep4(src, tile0, dstT3):
                t4 = load4(src, tile0 * 128)
                for g in range(4):
                    prep(t4, g, dstT3, (tile0 + g) * 128)

            def gemm_block(m, h, tail=False):
                """One [128, 2048] output row-chunk: x tile m vs w cols half h."""
                row = None if tail else rowb.tile(
                    [128, HALF], F32, name="row", tag="row")
                for nbl in range(4):
                    nb = h * 4 + nbl
                    pm = mmps.tile([128, 512], F32, name="pm", tag="pm")
                    for k in range(KC):
                        nc.tensor.matmul(
                            pm[:],
                            xT3[:, k, m * 128 : (m + 1) * 128],
                            wT3[:, k, nb * 512 : (nb + 1) * 512],
                            start=(k == 0), stop=(k == KC - 1))
                    if tail:
                        # finer eviction+DMA granularity to shrink the tail
                        ot = rowb.tile([128, 512], F32, name="ot", tag="ot",
                                       bufs=4)
                        copy_rr(ot[:], pm[:])
                        nc.sync.dma_start(
                            o[m * 128 : (m + 1) * 128,
                              nb * 512 : (nb + 1) * 512], ot[:])
                    else:
                        copy_rr(row[:, nbl * 512 : (nbl + 1) * 512], pm[:])
                if not tail:
                    nc.sync.dma_start(
                        o[m * 128 : (m + 1) * 128,
                          h * HALF : (h + 1) * HALF], row[:])

            # ---- emission schedule: keep PE fed from ~2us on ----
            prep4(x, 0, xT3)                   # x m0..3
            for b in range(4):                 # w j0..15 (column half 0)
                prep4(w, 4 * b, wT3)
            gemm_block(0, 0)
            gemm_block(1, 0)
            prep4(x, 4, xT3)                   # x m4..7
            gemm_block(2, 0)
            gemm_block(3, 0)
            prep4(w, 16, wT3)                  # w j16..19
            prep4(x, 8, xT3)                   # x m8..11
            for m in range(4, 8):
                gemm_block(m, 0)
            prep4(w, 20, wT3)                  # w j20..23
            prep4(x, 12, xT3)                  # x m12..15
            for m in range(8, 12):
                gemm_block(m, 0)
            prep4(w, 24, wT3)                  # w j24..27
            for m in range(12, 16):
                gemm_block(m, 0)
            prep4(w, 28, wT3)                  # w j28..31
            for m in range(16):
                gemm_block(m, 1, tail=(m == 15))

    nc.compile()
    return nc


def kernel(x: np.ndarray, weights: np.ndarray) -> np.ndarray:
    from concourse.bass_utils import run_bass_kernel_spmd

    if "nc" not in _cached:
        _cached["nc"] = _build()
    nc = _cached["nc"]

    x16 = np.ascontiguousarray(x, dtype=np.float16)
    w16 = np.ascontiguousarray(weights, dtype=np.float16)
    in_maps = [
        {"x": x16[i * BS : (i + 1) * BS], "weights": w16} for i in range(NCORES)
    ]
    res = run_bass_kernel_spmd(nc, in_maps, list(range(NCORES)))
    return np.concatenate([res.results[i]["out"] for i in range(NCORES)], axis=0)


# revision 31
# speedup vs baseline: 1.1311x; 1.1311x over previous
"""Cosine-similarity kernel (x[16384,512] vs weights[4096,512] -> [16384,4096])
on 8 Trainium2 NeuronCores, data-parallel over the x batch dim.

Per core: x shard [2048,512], full weights [4096,512], staged as fp16.
  out = normalize(x) @ normalize(w).T
w row norms are folded into the PE transposes by multiplying against
diag(1/||w_j||) instead of the identity, so the moving operand enters the
GEMM pre-normalized. x is transposed raw (no dependency chain before the PE
can start) and 1/||x_b|| is applied as a per-partition scale at PSUM
eviction. The GEMM runs as 8 column panels of 16 [128,512] output groups,
each group DMA'd out immediately so the DMA engines stay saturated and the
final tile drains right after the last matmul.
"""
import numpy as np

B, D, N = 16384, 512, 4096
NCORES = 8
BS = B // NCORES          # 2048 rows per core
MT = BS // 128            # 16 x row-tiles
NT = N // 128             # 32 w row-tiles
KC = D // 128             # 4 k-chunks
NB = N // 512             # 8 column panels

_cached = {}


def _build():
    import concourse.mybir as mybir
    import concourse.tile as tile
    from concourse import bacc
    from concourse.masks import make_identity

    F32, F16 = mybir.dt.float32, mybir.dt.float16
    AOP = mybir.AluOpType

    nc = bacc.Bacc(None, target_bir_lowering=False)
    x = nc.dram_tensor("x", [BS, D], F16, kind="ExternalInput")
    w = nc.dram_tensor("weights", [N, D], F16, kind="ExternalInput")
    o = nc.dram_tensor("out", [BS, N], F32, kind="ExternalOutput")

    with tile.TileContext(nc) as tc:
        with (
            tc.tile_pool(name="const", bufs=1) as const,
            tc.tile_pool(name="stage", bufs=9) as stage,
            tc.tile_pool(name="norm", bufs=6) as norm,
            tc.tile_pool(name="outs", bufs=10) as outs,
            tc.tile_pool(name="mmps", bufs=5, space="PSUM") as mmps,
            tc.tile_pool(name="trps", bufs=3, space="PSUM") as trps,
        ):
            ident = const.tile([128, 128], F16, name="ident")
            make_identity(nc, ident[:])

            # k-major transposed operands: [128(d%128), k, cols]
            wT = const.tile([128, KC * N], F16, name="wT")
            xT = const.tile([128, KC * BS], F16, name="xT")
            wT3 = wT[:].rearrange("p (k n) -> p k n", k=KC)
            xT3 = xT[:].rearrange("p (k n) -> p k n", k=KC)
            # 1/||x_row|| per x tile, applied at eviction
            rx = const.tile([128, MT], F32, name="rx")

            rr = [0]  # ACT/DVE round-robin for copies / scaled evictions

            import os

            # Warm the ACT Square/Sqrt function table before inputs arrive so
            # the 1.3us LoadActFuncSet doesn't land mid-pipeline.
            if os.environ.get("KWU", "1") == "1":
                wu = norm.tile([128, 1], F16, name="wu", tag="wu")
                wss = norm.tile([128, 1], F32, name="wss", tag="wss")
                nc.scalar.activation(
                    wu[:], ident[:, 0:1],
                    mybir.ActivationFunctionType.Square, accum_out=wss[:])
                wsq = norm.tile([128, 1], F32, name="wsq", tag="wsq")
                nc.scalar.sqrt(wsq[:], wss[:])

            def loadn(src, row0, n):
                """One DMA pulling n consecutive 128-row tiles."""
                t4 = stage.tile([128, 4 * D], F16, name="t4", tag="ld")
                dst = t4[:, 0 : n * D].rearrange("p (g d) -> p g d", g=n)
                srcap = src[row0 : row0 + n * 128, :].rearrange(
                    "(g p) d -> p g d", p=128
                )
                nc.sync.dma_start(dst, srcap)
                return t4

            def norms(t, alt, rout):
                """rout[128,1] = 1/||row|| for one [128, D] tile."""
                sq = stage.tile([128, D], F16, name="sq", tag="sq")
                ss = norm.tile([128, 1], F32, name="ss", tag="ss")
                if alt % 2 == 0:
                    nc.scalar.activation(
                        sq[:], t, mybir.ActivationFunctionType.Square,
                        accum_out=ss[:])
                else:
                    # tensor_tensor_reduce faults on hardware; square+reduce
                    nc.vector.tensor_mul(sq[:], t, t)
                    nc.vector.tensor_reduce(
                        ss[:], sq[:], mybir.AxisListType.XYZW, AOP.add)
                inv = norm.tile([128, 1], F32, name="inv", tag="inv")
                nc.vector.reciprocal(inv[:], ss[:])
                nc.scalar.sqrt(rout, inv[:])

            def transpose4(t, rhs, dstT3, col0):
                """PE-transpose one [128, D] tile against rhs into dstT3."""
                pt = trps.tile([128, KC * 128], F32, name="pt", tag="pt")
                for k in range(KC):
                    nc.tensor.matmul(
                        pt[:, k * 128 : (k + 1) * 128],
                        t[:, k * 128 : (k + 1) * 128], rhs,
                        start=True, stop=True)
                dst = dstT3[:, :, col0 : col0 + 128]
                src = pt[:].rearrange("p (k c) -> p k c", k=KC)
                if rr[0] % 2 == 0:
                    nc.scalar.copy(dst, src)
                else:
                    nc.vector.tensor_copy(dst, src)
                rr[0] += 1

            def w_prep(t4, g, j):
                """Normalize+transpose w tile j into wT."""
                t = t4[:, g * D : (g + 1) * D]
                rn = norm.tile([128, 1], F32, name="rn", tag="rn")
                norms(t, j, rn[:])
                dg = norm.tile([128, 128], F16, name="dg", tag="dg")
                if j % 2 == 0:
                    nc.vector.tensor_scalar_mul(dg[:], ident[:], rn[:])
                else:
                    nc.gpsimd.tensor_scalar_mul(dg[:], ident[:], rn[:])
                transpose4(t, dg[:], wT3, j * 128)

            def x_t(t4, g, m):
                """Transpose raw x tile m into xT (no chain dependencies)."""
                transpose4(t4[:, g * D : (g + 1) * D], ident[:], xT3, m * 128)

            def x_n(t4, g, m):
                """Stash 1/||x tile m row|| into rx (needed at evictions)."""
                norms(t4[:, g * D : (g + 1) * D], g, rx[:, m : m + 1])

            def gemm_group(m, nb, split=1):
                """One [128, 512] output tile: matmuls, scaled evict, DMA.

                split>1 chops the group into column pieces (each with its own
                PSUM tile, so pieces don't serialize on whole-tile WAR deps)
                to drain the final tile with minimal latency.
                """
                cw = 512 // split
                for s in range(split):
                    if split == 1:
                        pm = mmps.tile([128, 512], F32, name="pm", tag="pm")
                    else:
                        pm = trps.tile([128, KC * 128], F32, name="pq",
                                       tag="pt")
                    for k in range(KC):
                        nc.tensor.matmul(
                            pm[:, 0:cw],
                            xT3[:, k, m * 128 : (m + 1) * 128],
                            wT3[:, k, nb * 512 + s * cw : nb * 512 + (s + 1) * cw],
                            start=(k == 0), stop=(k == KC - 1))
                    ot = outs.tile([128, 512], F32, name="ot", tag="ot")
                    if rr[0] % 2 == 0:
                        nc.scalar.mul(ot[:, 0:cw], pm[:, 0:cw],
                                      rx[:, m : m + 1])
                    else:
                        nc.vector.tensor_scalar_mul(ot[:, 0:cw], pm[:, 0:cw],
                                                    rx[:, m : m + 1])
                    rr[0] += 1
                    nc.sync.dma_start(
                        o[m * 128 : (m + 1) * 128,
                          nb * 512 + s * cw : nb * 512 + (s + 1) * cw],
                        ot[:, 0:cw])

            # ---- emission schedule ----
            # w j0..3 as single-tile loads (shortest path through the prep
            # chains, norm engines alternating by j), then all x loads. x
            # transposes for b1..3 and all x norm chains are deferred into
            # panel 0 so they neither clog ACT/DVE ahead of the w chains nor
            # stall the PE on not-yet-landed x data.
            twj = [loadn(w, j * 128, 1) for j in range(4)]
            tx = [loadn(x, b * 4 * 128, 4) for b in range(4)]
            # Dummy matmuls on the identity bridge the ~4.7us until input
            # data lands, holding the PE p-state ramp so real work runs at
            # full clock from the first tile.
            for _ in range(0 if os.environ.get("KDUM", "1") == "0" else 40):
                pd = trps.tile([128, KC * 128], F32, name="pd", tag="pt")
                nc.tensor.matmul(pd[:, 0:128], ident[:], ident[:],
                                 start=True, stop=True)
            if os.environ.get("KWP", "1") == "1":
                for j in range(4):
                    w_prep(twj[j], 0, j)
            if os.environ.get("KXT", "1") == "1":
                for g in range(4):
                    x_t(tx[0], g, g)
            if os.environ.get("KXN", "1") == "1":
                for g in range(4):
                    x_n(tx[0], g, g)

            # Column panels; w batch nb+1 is DMA'd a panel ahead and its
            # compute is emitted mid-panel so the chain hides under the GEMM.
            # Panel 0 also absorbs the remaining x transposes + norm chains.
            import os
            kpanels = int(os.environ.get("KPANELS", str(NB)))
            wnext = [loadn(w, 4 * 128, 4)]
            for nb in range(kpanels):
                t4w = wnext[0] if nb < NB - 1 else None
                for m in range(MT):
                    if nb == 0 and m in (1, 3, 5):
                        b = (m + 1) // 2
                        for g in range(4):
                            x_t(tx[b], g, 4 * b + g)
                    if nb == 0 and m in (2, 4, 6):
                        b = m // 2
                        for g in range(4):
                            x_n(tx[b], g, 4 * b + g)
                    if m == 8 and nb < NB - 1:
                        for g in range(4):
                            w_prep(t4w, g, 4 * (nb + 1) + g)
                    gemm_group(m, nb)
                if nb + 2 < NB:
                    wnext[0] = loadn(w, 4 * (nb + 2) * 128, 4)

    nc.compile()
    return nc


def kernel(x: np.ndarray, weights: np.ndarray) -> np.ndarray:
    from concourse.bass_utils import run_bass_kernel_spmd

    if "nc" not in _cached:
        _cached["nc"] = _build()
    nc = _cached["nc"]

    x16 = np.ascontiguousarray(x, dtype=np.float16)
    w16 = np.ascontiguousarray(weights, dtype=np.float16)
    in_maps = [
        {"x": x16[i * BS : (i + 1) * BS], "weights": w16} for i in range(NCORES)
    ]
    res = run_bass_kernel_spmd(nc, in_maps, list(range(NCORES)))
    return np.concatenate([res.results[i]["out"] for i in range(NCORES)], axis=0)
